# revision 3
# baseline (speedup 1.0000x reference)
"""Partial-FC style sharded loss kernel for trn2 (8 NeuronCores).

Math (reference):
  cosine = clip(normalize(x) @ normalize(W).T)          (N, C)
  raw    = x @ W.T ; output = cosine with label col set to raw
  loss   = mean(weights * (-log_softmax(output)[label])) with
           weights = lam * (ms*(1-cosine)+2) + (1-lam)
  prec1  = 100 * mean(argmax(output) == labels)

Device work (the N*C-scale part), class-sharded across 8 cores:
  cos_block = xn @ wn_shard.T via PE (bf16 in, fp32 PSUM)
  per row:  sum_c exp(cos)  (ACT exp + accum_out)
            max_c cos       (DVE reduce_max from PSUM)
Everything O(N*D)/O(C*D) (norms, label column, sum_c cosine via linearity)
is exact host-side numpy; the final scalar combine is host fp64.
"""

import numpy as np
import ml_dtypes

N, D, C = 1024, 512, 100000
NCORES = 8
CPC = C // NCORES          # classes per core: 12500
CW = 500                   # class block width on device
NCB = CPC // CW            # 25 c-blocks
NT = N // 128              # 8 n-tiles
KD = D // 128              # 4 contraction chunks
T_ALPHA = 0.98
EPS = 0.001

_PROGRAM = None


def _split_multi_waits(nc, mybir):
    # The walrus build in this container rejects >1 sem-wait per instruction
    # ("Too many sync wait commands"); move extra waits onto same-engine NoOps
    # placed immediately before the owning instruction.
    n_split = 0
    for bb in nc.m.functions[0].blocks:
        new_insts = []
        for inst in bb.instructions:
            si = inst.sync_info
            if si is not None and si.on_wait and len(si.on_wait) > 1:
                waits = list(si.on_wait)
                for i, w in enumerate(waits[:-1]):
                    nop = mybir.InstNoOp(
                        name=f"waitsplit_{inst.name}_{i}",
                        engine=inst.engine,
                        ins=[], outs=[],
                        sync_info=mybir.SyncInfo(on_wait=[w], on_update=[]),
                    )
                    nc.register_instruction(nop)
                    new_insts.append(nop)
                    n_split += 1
                si.on_wait = waits[-1:]
            new_insts.append(inst)
        bb.instructions[:] = new_insts
    return n_split


def _build_program(repeat=1, psum_bufs=6, wn_bufs=3, scr_bufs=3):
    import concourse.bass as bass
    import concourse.mybir as mybir
    import concourse.tile as tile

    nc = bass.Bass()
    xn_in = nc.dram_tensor("xnT", [D, N], mybir.dt.bfloat16, kind="ExternalInput")
    wn_in = nc.dram_tensor("wnT", [D, CPC], mybir.dt.bfloat16, kind="ExternalInput")
    se_out = nc.dram_tensor("sumexp", [N, NCB], mybir.dt.float32, kind="ExternalOutput")
    mx_out = nc.dram_tensor("maxexp", [N, 1], mybir.dt.float32, kind="ExternalOutput")

    with tile.TileContext(nc) as tc:
        with (
            tc.tile_pool(name="xn", bufs=1) as xn_pool,
            tc.tile_pool(name="wn", bufs=wn_bufs) as wn_pool,
            tc.tile_pool(name="scratch", bufs=scr_bufs) as scr_pool,
            tc.tile_pool(name="col", bufs=1) as col_pool,
            tc.tile_pool(name="ps", bufs=psum_bufs, space="PSUM") as ps_pool,
        ):
            xn_sb = xn_pool.tile([128, KD * N], mybir.dt.bfloat16)
            nc.sync.dma_start(
                xn_sb[:].rearrange("p (k n) -> p k n", k=KD),
                xn_in.ap().rearrange("(k p) n -> p k n", p=128),
            )
            se_cols = [col_pool.tile([128, NCB], mybir.dt.float32, tag=f"se{i}", name=f"se{i}")
                       for i in range(NT)]
            mx_cols = [col_pool.tile([128, 1], mybir.dt.float32, tag=f"mx{i}", name=f"mx{i}")
                       for i in range(NT)]
            mxaccs = [col_pool.tile([128, CW], mybir.dt.bfloat16, tag=f"mxa{i}", name=f"mxa{i}")
                      for i in range(NT)]

            def body(_iv=None):
                for nt in range(NT):
                    nc.gpsimd.memset(mxaccs[nt][:], 0.0)
                for cb in range(NCB):
                    w_sb = wn_pool.tile([128, KD * CW], mybir.dt.bfloat16, tag="w", name="w_sb")
                    nc.sync.dma_start(
                        w_sb[:].rearrange("p (k c) -> p k c", k=KD),
                        wn_in.ap()[:, cb * CW:(cb + 1) * CW].rearrange("(k p) c -> p k c", p=128),
                    )
                    for nt in range(NT):
                        ps = ps_pool.tile([128, CW], mybir.dt.float32, tag="ps", name="ps")
                        for k in range(KD):
                            nc.tensor.matmul(
                                ps[:],
                                lhsT=xn_sb[:, k * N + nt * 128: k * N + (nt + 1) * 128],
                                rhs=w_sb[:, k * CW:(k + 1) * CW],
                                start=(k == 0), stop=(k == KD - 1),
                            )
                        scr = scr_pool.tile([128, CW], mybir.dt.bfloat16, tag="scr", name="scr")
                        nc.scalar.activation(scr[:], ps[:], mybir.ActivationFunctionType.Exp,
                                             accum_out=se_cols[nt][:, cb:cb + 1])
                        nc.vector.tensor_max(mxaccs[nt][:], mxaccs[nt][:], scr[:])
                for nt in range(NT):
                    nc.vector.reduce_max(mx_cols[nt][:], mxaccs[nt][:],
                                         axis=mybir.AxisListType.X)

            if repeat == 1:
                body()
            else:
                with tc.For_i(0, repeat, 1) as _i:
                    body(_i)
            for nt in range(NT):
                nc.sync.dma_start(se_out.ap()[nt * 128:(nt + 1) * 128, :], se_cols[nt][:])
                nc.sync.dma_start(mx_out.ap()[nt * 128:(nt + 1) * 128, :], mx_cols[nt][:])

    _split_multi_waits(nc, mybir)
    return nc


def _get_program():
    global _PROGRAM
    if _PROGRAM is None:
        _PROGRAM = _build_program()
    return _PROGRAM


def _run_device(xnT_bf16, wnT_bf16_full, trace=False):
    from concourse.bass_utils import run_bass_kernel_spmd

    nc = _get_program()
    in_maps = [
        {"xnT": xnT_bf16,
         "wnT": np.ascontiguousarray(wnT_bf16_full[:, c * CPC:(c + 1) * CPC])}
        for c in range(NCORES)
    ]
    res = run_bass_kernel_spmd(nc, in_maps, core_ids=list(range(NCORES)), trace=trace)
    se = np.stack([res.results[c]["sumexp"] for c in range(NCORES)])  # (8, N, NCB)
    mx = np.stack([res.results[c]["maxexp"] for c in range(NCORES)])   # (8, N, 1)
    return se, mx, res


def kernel(x, weight, batch_mean, labels, ith_iter, total_iter, _trace=False,
           _return_res=False):
    x = np.asarray(x, dtype=np.float32)
    weight = np.asarray(weight, dtype=np.float32)
    batch_mean = np.asarray(batch_mean, dtype=np.float32)
    labels = np.asarray(labels).astype(np.int64)

    x64 = x.astype(np.float64)
    norms = np.linalg.norm(x64, axis=1)                      # (N,)
    safe_norms = np.clip(norms, 0.001, 200.0)
    mean = safe_norms.mean()
    new_batch_mean = mean * T_ALPHA + (1.0 - T_ALPHA) * float(batch_mean[0])
    ms = np.where(safe_norms > new_batch_mean, 1.0, -1.0)    # (N,)

    xn = x64 / np.maximum(norms, 1e-12)[:, None]             # (N, D) f64
    wnorms = np.linalg.norm(weight.astype(np.float64), axis=1)   # (C,)
    wn32 = (weight / np.maximum(wnorms, 1e-12)[:, None].astype(np.float32))  # (C, D) f32

    # sum_c cosine per row via linearity (exact to fp64 roundoff)
    s = wn32.sum(axis=0, dtype=np.float64)                   # (D,)
    rowsum_cos = xn @ s                                      # (N,)

    # label column quantities, exact
    wl = weight[labels].astype(np.float64)                   # (N, D)
    raw_label = (x64 * wl).sum(axis=1)                       # (N,)
    nwl = np.maximum(wnorms[labels], 1e-12)
    cos_label = np.clip(raw_label / (np.maximum(norms, 1e-12) * nwl),
                        -1.0 + EPS, 1.0 - EPS)

    # device: sharded cosine GEMM + per-row sum-exp / max
    xnT = np.ascontiguousarray(xn.T).astype(ml_dtypes.bfloat16)      # (D, N)
    wnT = np.ascontiguousarray(wn32.T).astype(ml_dtypes.bfloat16)    # (D, C)
    se, mx, res = _run_device(xnT, wnT, trace=_trace)

    S_cos = se.sum(axis=(0, 2), dtype=np.float64)            # (N,)
    S = S_cos - np.exp(cos_label) + np.exp(raw_label)
    logZ = np.log(S)
    ce = logZ - raw_label                                    # (N,)

    lam = float(ith_iter) / float(total_iter)
    wrow = lam * (ms * (C - rowsum_cos) + 2.0 * C) + (1.0 - lam) * C
    loss = np.float32((ce * wrow).sum() / (N * C))

    # prec1: device max includes the label-position cosine; recheck rows where
    # bf16 noise or the label-is-max case could flip argmax-vs-label.
    maxcos = np.log(mx.max(axis=(0, 2)))                     # (N,) from bf16 exp
    correct = raw_label > maxcos
    suspect = (np.abs(raw_label - maxcos) < 8e-3) | (cos_label >= maxcos - 8e-3)
    if suspect.any():
        xn32 = xn.astype(np.float32)
        for n in np.nonzero(suspect)[0]:
            cosr = np.clip(xn32[n] @ wn32.T, -1.0 + EPS, 1.0 - EPS)
            out_row = cosr.astype(np.float64)
            out_row[labels[n]] = raw_label[n]
            correct[n] = out_row.argmax() == labels[n]
    prec1 = np.float32(correct.mean() * 100.0)

    if _return_res:
        return (loss, prec1), res
    return (loss, prec1)
